# revision 8
# baseline (speedup 1.0000x reference)
"""Trainium2 Bass kernel: 16-head attention (S=1024, hidden=1024) + output
linear, data-parallel over the batch dimension (8 batch elements -> 8 cores).

Contract: kernel(**inputs) takes the FULL unsharded inputs of
nn_Attention_83915071029891 and returns the FULL (8, 1024, 1024) f32 output.

Per-core algorithm, v2 (ACT-bound pipeline with FC interleave):
  The steady-state cadence is set by the ScalarE exp (softmax numerator) --
  ~1ns/lane-element, ~997ns per [128,1024] tile -- so everything else is
  arranged to hide under it:
    - q-pass (512 q) outer, head-pairs inner; all of K/V/Q resident in SBUF.
    - per (pair, qpass, ktile): row-packed QK^T pair (heads A|B on PE row
      groups), one exp ACT over both heads, two PSUM-accumulated AV matmuls
      (v augmented with a ones row for the softmax denominator).
    - normalization: per-pair denominators -> DVE reciprocal -> DMA broadcast
      -> DVE multiply into outT (no PE broadcast matmuls, no PE-queue stalls).
    - the output linear for q-pass n-1 is interleaved one matmul per
      iteration slot during q-pass n, filling the PE slack under ACT.
    - kernel-start warmup: dummy matmuls + a dummy exp pre-warm the PE HAM
      clock gate (1.2 -> 2.4 GHz takes ~3.4us of sustained PE activity) and
      the ACT exp table while the input DMAs stream in.
"""

import sys

for _p in ("/opt/trn_rl_repo", "/root/.axon_site/_ro/trn_rl_repo"):
    if _p not in sys.path:
        sys.path.append(_p)

from contextlib import ExitStack

import numpy as np

import bass_rust
import concourse.bass as bass
import concourse.mybir as mybir
import concourse.tile as tile
from concourse.vector_clock import ScopedClock

F32 = mybir.dt.float32
AF = mybir.ActivationFunctionType

N_CORES = 8
_MAX_CTRL_WAITS = 1
ROWPACK_DEP = True


def _patched_drain_and_barrier(self, tick_clock, wait_clock):
    """Tile's kernel-tail Drain aggregates one sem wait per outstanding proc,
    but walrus CoreV3 codegen only has one sync-wait slot on CTRL ops -- split
    the waits across a chain of SP drain instructions."""
    nc = self.nc
    drain_inst = nc.sync.drain()
    wait_clock.add_sem_waits(
        drain_inst.ins, ScopedClock({None: tick_clock.global_clock})
    )
    si = drain_inst.ins.sync_info
    if si is not None and si.on_wait and len(si.on_wait) > _MAX_CTRL_WAITS:
        waits = list(si.on_wait)
        drain_inst.ins.sync_info = bass_rust.SyncInfo(
            on_wait=waits[:_MAX_CTRL_WAITS], on_update=list(si.on_update or [])
        )
        for i in range(_MAX_CTRL_WAITS, len(waits), _MAX_CTRL_WAITS):
            extra = nc.sync.drain()
            extra.ins.sync_info = bass_rust.SyncInfo(
                on_wait=waits[i : i + _MAX_CTRL_WAITS], on_update=[]
            )

    nc.all_engine_barrier()
    assert self.sems is not None
    popped = nc._tile_sem_poison_stack.pop()
    assert popped is self._sem_poison
    nc.clear_and_free_semaphores(list(self.sems.allocated().values()))
    nc.all_engine_barrier()


tile.TileContext._drain_and_barrier = _patched_drain_and_barrier


def _split_excess_waits(nc, max_waits=_MAX_CTRL_WAITS):
    """walrus CoreV3 setupSyncWait only has one sync-wait slot per
    instruction; hoist excess sem waits onto same-engine NoOp carriers
    inserted immediately before the over-limit instruction."""
    ctr = [0]

    def carrier(engine, waits):
        ctr[0] += 1
        nop = mybir.InstNoOp(name=f"I-waitc-{ctr[0]}", ins=[], outs=[])
        nop.engine = engine
        nop.sync_info = bass_rust.SyncInfo(on_wait=waits, on_update=[])
        return nop

    for fn in nc.m.functions:
        for blk in fn.blocks:
            il = blk.instructions
            newl = []
            changed = False
            for inst in il:
                si = inst.sync_info
                nw = len(si.on_wait) if si and si.on_wait else 0
                if nw > max_waits:
                    waits = list(si.on_wait)
                    for i in range(max_waits, len(waits), max_waits):
                        newl.append(carrier(inst.engine, waits[i : i + max_waits]))
                    inst.sync_info = bass_rust.SyncInfo(
                        on_wait=waits[:max_waits], on_update=list(si.on_update or [])
                    )
                    changed = True
                newl.append(inst)
            if changed:
                il.clear()
                il.extend(newl)
                assert len(blk.instructions) == len(newl), (
                    "block instruction list is not a live reference"
                )


def build_kernel(S=1024, HEADS=16, mm_dtype="bf16", split_waits=True):
    """Trace the per-core Bass program. DRAM io: qT,kT,vaug,fc_wT,fc_b,wones -> y."""
    HD = 64
    H = HEADS * HD
    KT = S // 128
    PAIRS = HEADS // 2
    ITILES = H // 128
    VW = HD + 1
    SCALE = 1.0 / float(H) ** 0.5
    NW = 512
    NPASS = S // NW
    QT_PER_PASS = NW // 128
    WARM_MMS = 10

    nc = bass.Bass(trn_type="TRN2")

    MMDT = {"bf16": mybir.dt.bfloat16, "f32": F32}[mm_dtype]

    qT = nc.dram_tensor("qT", [H, S], MMDT, kind="ExternalInput").ap()
    kT = nc.dram_tensor("kT", [H, S], MMDT, kind="ExternalInput").ap()
    vaug = nc.dram_tensor("vaug", [HEADS, 128, KT * VW], MMDT, kind="ExternalInput").ap()
    fc_wT = nc.dram_tensor("fc_wT", [H, H], MMDT, kind="ExternalInput").ap()
    fc_b = nc.dram_tensor("fc_b", [1, H], F32, kind="ExternalInput").ap()
    wones = nc.dram_tensor("wones", [64, NW], MMDT, kind="ExternalInput").ap()
    y = nc.dram_tensor("y", [S, H], F32, kind="ExternalOutput").ap()

    with tile.TileContext(nc) as tc:
        with ExitStack() as ctx:
            big = ctx.enter_context(tc.tile_pool(name="big", bufs=1))
            kp = ctx.enter_context(tc.tile_pool(name="kp", bufs=PAIRS))
            qp = ctx.enter_context(tc.tile_pool(name="qp", bufs=PAIRS * NPASS))
            vp = ctx.enter_context(tc.tile_pool(name="vp", bufs=PAIRS))
            at = ctx.enter_context(tc.tile_pool(name="at", bufs=6))
            st = ctx.enter_context(tc.tile_pool(name="st", bufs=2))
            rc = ctx.enter_context(tc.tile_pool(name="rc", bufs=2))
            yp = ctx.enter_context(tc.tile_pool(name="yp", bufs=2))
            # PSUM budget (8 banks): sAB 2x2, po_A+po_B 2, fc accum 2x1
            psS = ctx.enter_context(tc.tile_pool(name="psS", bufs=2, space="PSUM"))
            psO = ctx.enter_context(tc.tile_pool(name="psO", bufs=1, space="PSUM"))
            psF = ctx.enter_context(tc.tile_pool(name="psF", bufs=2, space="PSUM"))

            # --- warmup: tiny load, then dummy MMs to trip the HAM clock
            # gate to 2.4 GHz and a dummy exp to pull the ACT table load
            # forward, all while the attention inputs stream in.
            wones_sb = big.tile([64, NW], MMDT, tag="wones")
            nc.sync.dma_start(out=wones_sb[:, :], in_=wones[:, :])
            wexp = big.tile([1, 64], MMDT, tag="wexp")
            nc.scalar.activation(wexp[:, :], wones_sb[0:1, 0:64], AF.Exp, scale=SCALE)
            for w in range(WARM_MMS):
                warm = psF.tile([128, NW], F32, tag="fc")
                nc.tensor.matmul(
                    warm[0:64, :], wones_sb[:, 0:64], wones_sb[:, :],
                    start=True, stop=True,
                )

            # --- resident input tiles; per-(pair[,pass]) so nothing ever
            # waits on an unrelated DMA chunk.
            kts, qts, vas, vbs = [], {}, [], []
            fcw_sb = big.tile([128, ITILES * H], MMDT, tag="fcw")
            fcb_sb = big.tile([128, H], F32, tag="fcb")
            outTs = [
                big.tile([128, ITILES * NW], MMDT, tag=f"outT{n}", name=f"outT{n}")
                for n in range(NPASS)
            ]

            def load_pair(p):
                psl = slice(128 * p, 128 * (p + 1))
                kTp = kp.tile([128, S], MMDT, tag="kT")
                nc.sync.dma_start(out=kTp[:, :], in_=kT[psl, :])
                qTp = qp.tile([128, NW], MMDT, tag="qT")
                nc.sync.dma_start(out=qTp[:, :], in_=qT[psl, 0:NW])
                vA = vp.tile([128, KT * VW], MMDT, tag="vA")
                nc.sync.dma_start(out=vA[:, :], in_=vaug[2 * p])
                vB = vp.tile([128, KT * VW], MMDT, tag="vB")
                nc.sync.dma_start(out=vB[:, :], in_=vaug[2 * p + 1])
                kts.append(kTp)
                qts[(p, 0)] = qTp
                vas.append(vA)
                vbs.append(vB)

            for p in range(PAIRS):
                load_pair(p)
            for n in range(1, NPASS):
                for p in range(PAIRS):
                    psl = slice(128 * p, 128 * (p + 1))
                    qTp = qp.tile([128, NW], MMDT, tag="qT")
                    nc.sync.dma_start(
                        out=qTp[:, :], in_=qT[psl, NW * n : NW * (n + 1)]
                    )
                    qts[(p, n)] = qTp
            # fc weights go at the tail of the SP-ring input queue (not
            # needed until the FC interleave ~half-way in); the GPSIMD SWDGE
            # ring is reserved for the small latency-sensitive reciprocal
            # broadcasts so they never sit behind megabyte-scale loads.
            for i in range(ITILES):
                nc.sync.dma_start(
                    out=fcw_sb[:, H * i : H * (i + 1)],
                    in_=fc_wT[128 * i : 128 * (i + 1), :],
                )
            nc.gpsimd.dma_start(
                out=fcb_sb[:, :], in_=fc_b.unsqueeze(1).broadcast_to((1, 128, H))
            )

            # --- deferred normalization: per (pair, pass), emitted during the
            # NEXT pair so the DVE muls never stall the pipeline on a fresh
            # reciprocal chain.
            def emit_norm(job):
                p_, n_, stA, stB, rbAB = job
                osl = slice(NW * p_, NW * (p_ + 1))
                nc.vector.tensor_mul(
                    outTs[n_][0:64, osl], stA[0:64, :], rbAB[:, 0:NW]
                )
                stg = rc.tile([64, NW], MMDT, tag="stg")
                nc.vector.tensor_mul(stg[:, :], stB[0:64, :], rbAB[:, NW : 2 * NW])
                nc.gpsimd.dma_start(out=outTs[n_][64:128, osl], in_=stg[:, :])

            # --- FC jobs for pass n: a flat list of closures, paced into the
            # iteration slots of pass n+1 (or the tail for the last pass).
            def fc_jobs_for_pass(n):
                jobs = []
                outT = outTs[n]
                for j in range(QT_PER_PASS):
                    for oh in range(2):
                        py = [None]

                        def start_half(py=py):
                            py[0] = psF.tile([128, 512], F32, tag="fc", name="pyfc")

                        for i in range(ITILES):
                            def mm(i=i, j=j, oh=oh, py=py, pre=(None if i else start_half)):
                                if pre is not None:
                                    pre()
                                nc.tensor.matmul(
                                    py[0][:, :],
                                    outT[:, NW * i + 128 * j : NW * i + 128 * (j + 1)],
                                    fcw_sb[:, H * i + 512 * oh : H * i + 512 * (oh + 1)],
                                    start=(i == 0), stop=(i == ITILES - 1),
                                )
                            jobs.append(mm)

                        def fin(n=n, j=j, oh=oh, py=py):
                            ysb = yp.tile([128, 512], F32, tag="ysb")
                            nc.vector.tensor_add(
                                ysb[:, :], py[0][:, :],
                                fcb_sb[:, 512 * oh : 512 * (oh + 1)],
                            )
                            qg = NW * n + 128 * j
                            nc.sync.dma_start(
                                out=y[qg : qg + 128, 512 * oh : 512 * (oh + 1)],
                                in_=ysb[:, :],
                            )
                        jobs.append(fin)
                return jobs

            pending_norm = []
            fcq = []

            for n in range(NPASS):
                slots_left = PAIRS * KT
                for p in range(PAIRS):
                    kTp, qTp = kts[p], qts[(p, n)]
                    vA, vB = vas[p], vbs[p]
                    if pending_norm:
                        emit_norm(pending_norm.pop(0))
                    po_A = psO.tile([VW, NW], F32, tag="oA")
                    po_B = psO.tile([VW, NW], F32, tag="oB")
                    for t in range(KT):
                        ksl = slice(128 * t, 128 * (t + 1))
                        sAB = psS.tile([128, 2 * NW], F32, tag="s")
                        mmA = nc.tensor.matmul(
                            sAB[:, 0:NW], kTp[0:64, ksl], qTp[0:64, :],
                            start=True, stop=True,
                        )
                        mmB = nc.tensor.matmul(
                            sAB[:, NW : 2 * NW], kTp[64:128, ksl], qTp[64:128, :],
                            start=True, stop=True,
                        )
                        # keep the K=64 row-group pair adjacent on PE so the
                        # two half-array matmuls run concurrently
                        if ROWPACK_DEP:
                            tile.add_dep_helper(
                                mmB.ins, mmA.ins, sync=False, reason="rowpack"
                            )
                        aAB = at.tile([128, 2 * NW], MMDT, tag="a")
                        nc.scalar.activation(aAB[:, :], sAB[:, :], AF.Exp, scale=SCALE)
                        vsl = slice(VW * t, VW * (t + 1))
                        nc.tensor.matmul(
                            po_A[:, :], vA[:, vsl], aAB[:, 0:NW],
                            start=(t == 0), stop=(t == KT - 1),
                        )
                        nc.tensor.matmul(
                            po_B[:, :], vB[:, vsl], aAB[:, NW : 2 * NW],
                            start=(t == 0), stop=(t == KT - 1),
                        )
                        # pace the previous pass's FC work into this slot's
                        # PE slack (skip the first few slots: their outT
                        # chunks' norms are still in flight)
                        slot = p * KT + t
                        slots_left -= 1
                        if fcq and slot >= 4:
                            take = -(-len(fcq) // max(slots_left, 1))
                            for _ in range(min(take, 3, len(fcq))):
                                fcq.pop(0)()

                    # evacuate the accumulators (frees the po PSUM banks for
                    # the next pair) and build the reciprocal broadcast
                    stA = st.tile([VW, NW], F32, tag="stA")
                    nc.vector.tensor_copy(stA[:, :], po_A[:, :])
                    stB = st.tile([VW, NW], F32, tag="stB")
                    nc.vector.tensor_copy(stB[:, :], po_B[:, :])
                    # denominators: reshape both heads' den rows onto 128
                    # partitions so the iterative RECIPROCAL walks few
                    # elements per lane
                    denP = rc.tile([128, NW // 64], F32, tag="denP")
                    nc.gpsimd.dma_start(
                        out=denP[0:64, :],
                        in_=stA[64:65, :].rearrange("p (a b) -> p a b", b=NW // 64),
                    )
                    nc.gpsimd.dma_start(
                        out=denP[64:128, :],
                        in_=stB[64:65, :].rearrange("p (a b) -> p a b", b=NW // 64),
                    )
                    recP = rc.tile([128, NW // 64], F32, tag="recP")
                    nc.vector.reciprocal(recP[:, :], denP[:, :])
                    recAB = rc.tile([1, 2 * NW], F32, tag="recAB")
                    nc.gpsimd.dma_start(
                        out=recAB[:, :].rearrange("p (a b) -> p a b", b=NW // 64),
                        in_=recP[:, :],
                    )
                    # broadcast the per-q reciprocals to 64 partitions on the
                    # (idle) gpsimd DMA ring; both heads in one transfer
                    rbAB = rc.tile([64, 2 * NW], F32, tag="rbAB")
                    nc.gpsimd.dma_start(
                        out=rbAB[:, :],
                        in_=recAB.unsqueeze(1).broadcast_to((1, 64, 2 * NW)),
                    )
                    pending_norm.append((p, n, stA, stB, rbAB))

                # next pass's FC work (paced into the following pass)
                fcq.extend(fc_jobs_for_pass(n))

            # tail: flush the last norms, then whatever FC work remains
            # (the final q-pass's linear layer)
            while pending_norm:
                emit_norm(pending_norm.pop(0))
            while fcq:
                fcq.pop(0)()

    if split_waits:
        _split_excess_waits(nc)
    return nc


def prep_core_inputs(q_n, k_n, v_n, fc_wT, fc_b1, HEADS=16, mm_dtype="bf16"):
    """Host-side layout prep for one batch element."""
    import ml_dtypes

    cast = (lambda a: a.astype(ml_dtypes.bfloat16)) if mm_dtype == "bf16" else (lambda a: a)
    HD = 64
    S, H = q_n.shape
    KT = S // 128
    qT = np.ascontiguousarray(q_n.T)
    kT = np.ascontiguousarray(k_n.T)
    v4 = v_n.reshape(KT, 128, HEADS, HD)  # [t, p, h, c]
    vaug = np.empty((HEADS, 128, KT, HD + 1), dtype=np.float32)
    vaug[..., :HD] = v4.transpose(2, 1, 0, 3)
    vaug[..., HD] = 1.0
    return {
        "wones": cast(np.ones((64, 512), dtype=np.float32)),
        "qT": cast(qT),
        "kT": cast(kT),
        "vaug": cast(np.ascontiguousarray(vaug.reshape(HEADS, 128, KT * (HD + 1)))),
        "fc_wT": cast(fc_wT),
        "fc_b": fc_b1,
    }


MM_DTYPE = "bf16"

_CACHED_NC = None


def _get_nc():
    global _CACHED_NC
    if _CACHED_NC is None:
        _CACHED_NC = build_kernel(mm_dtype=MM_DTYPE)
    return _CACHED_NC


def make_in_maps(key, value, query, fc_w, fc_b):
    key = np.asarray(key, dtype=np.float32)
    value = np.asarray(value, dtype=np.float32)
    query = np.asarray(query, dtype=np.float32)
    fc_w = np.asarray(fc_w, dtype=np.float32)
    fc_b = np.asarray(fc_b, dtype=np.float32)
    N, S, H = query.shape
    fc_wT = np.ascontiguousarray(fc_w.T)
    fc_b1 = np.ascontiguousarray(fc_b.reshape(1, H))
    return [
        prep_core_inputs(query[n], key[n], value[n], fc_wT, fc_b1, mm_dtype=MM_DTYPE)
        for n in range(N)
    ]


def run_on_device(in_maps):
    from concourse.bass_utils import run_bass_kernel_spmd

    nc = _get_nc()
    res = run_bass_kernel_spmd(nc, in_maps, list(range(N_CORES)))
    return np.stack([res.results[i]["y"] for i in range(N_CORES)], axis=0)


def kernel(key, value, query, fc_w, fc_b):
    """Full inputs in, full output out. Shards batch N=8 across 8 cores."""
    in_maps = make_in_maps(key, value, query, fc_w, fc_b)
    return run_on_device(in_maps)
